# revision 7
# baseline (speedup 1.0000x reference)
"""Trainium2 Bass kernel for a dense transformer block with linear attention.

Reference computation (per batch b, sequence s):
    h   = linear_attention(LN1(x)); z = LN2(x + h)
    out = x + GeLU(z @ W1 + b1) @ W2 + b2

Sharding: tokens (B*S = 16384) split 8 ways -> 2048 tokens/core; each core
holds half the sequence of one batch element. The only cross-token coupling
is the sequence-dim softmax of k and the kv-einsum, handled by a pairwise
AllReduce of per-head [64, 65] (k^T v | sum exp k) statistics between the
two cores of each batch element.

LN gamma/beta are folded into the following matmul weights on the host.
Softmax max-subtraction is skipped (inputs are O(1); exp is safe in f32).
"""
import numpy as np
import ml_dtypes

import concourse.bass as bass
import concourse.tile as tile
from concourse import bacc, mybir
from concourse import bass_utils
from concourse.masks import make_identity

F32 = mybir.dt.float32
F32R = mybir.dt.float32r
BF16 = mybir.dt.bfloat16
AX = mybir.AxisListType
ALU = mybir.AluOpType
ACTF = mybir.ActivationFunctionType

N_CORES = 8
T = 2048          # tokens per core
D = 1024
H = 16            # heads
DH = 64
F3 = 3 * D        # qkv width
DFF = 4096
NT = T // 128     # 16 token tiles
ND = D // 128     # 8 d-slices
NF = DFF // 128   # 32 ff tiles
EPS = 1e-5

_CACHE = {}


def _r(ap):
    return ap.bitcast(F32R)


def build_kernel(has_bqkv, has_bout, has_b2):
    nc = bacc.Bacc("TRN2", target_bir_lowering=False, debug=False,
                   enable_asserts=True, num_devices=N_CORES)

    x_d = nc.dram_tensor("x", [T, D], F32, kind="ExternalInput").ap()
    wqkv_d = nc.dram_tensor("wqkv", [D, F3], BF16, kind="ExternalInput").ap()
    wout_d = nc.dram_tensor("wout", [D, D], F32R, kind="ExternalInput").ap()
    w1_d = nc.dram_tensor("w1", [D, DFF], BF16, kind="ExternalInput").ap()
    w2_d = nc.dram_tensor("w2", [DFF, D], BF16, kind="ExternalInput").ap()
    bqkv_d = nc.dram_tensor("bqkv", [1, F3], F32, kind="ExternalInput").ap()
    ident_d = nc.dram_tensor("ident", [128, 128], F32R, kind="ExternalInput").ap()
    ones_d = nc.dram_tensor("ones8", [1, 8], F32, kind="ExternalInput").ap()
    bout_d = nc.dram_tensor("bout", [1, D], F32, kind="ExternalInput").ap()
    b1_d = nc.dram_tensor("b1", [NF, 128], F32, kind="ExternalInput").ap()
    b2_d = nc.dram_tensor("b2", [1, D], F32, kind="ExternalInput").ap()
    out_d = nc.dram_tensor("out", [T, D], F32, kind="ExternalOutput").ap()

    with tile.TileContext(nc) as tc:
        with tc.tile_pool(name="const", bufs=1) as cst, \
             tc.tile_pool(name="persist", bufs=1) as per, \
             tc.tile_pool(name="dram", bufs=1, space="DRAM") as dramp:
            ident = cst.tile([128, 128], F32R)
            nc.sync.dma_start(ident[:], ident_d[:])
            ones8 = cst.tile([128, 8], F32)
            nc.sync.dma_start(ones8[:], ones_d.partition_broadcast(128).squeeze(1))
            eps_t = cst.tile([128, 1], F32)
            nc.vector.memset(eps_t[:], EPS)
            b1_cols = cst.tile([128, NF], F32)
            nc.sync.dma_start(b1_cols[:], b1_d.transpose([1, 0]))
            bqkv_bc = bout_bc = b2_bc = None
            if has_bqkv:
                bqkv_bc = cst.tile([128, F3], F32)
                nc.sync.dma_start(bqkv_bc[:],
                                  bqkv_d.partition_broadcast(128).squeeze(1))
            if has_bout:
                bout_bc = cst.tile([128, D], F32)
                nc.sync.dma_start(bout_bc[:],
                                  bout_d.partition_broadcast(128).squeeze(1))
            if has_b2:
                b2_bc = cst.tile([128, D], F32)
                nc.sync.dma_start(b2_bc[:],
                                  b2_d.partition_broadcast(128).squeeze(1))

            qT_all = per.tile([128, NT, ND, 128], BF16)    # 4 MB, softmaxed q^T
            kv_red = per.tile([128, ND, DH + 1], F32)      # allreduced kv | sumexp
            kvn = per.tile([128, ND, DH], BF16)            # normalized kv

            kv_in = dramp.tile([128, ND, DH + 1], F32)
            kv_out = dramp.tile([128, ND, DH + 1], F32)
            zT_dram = dramp.tile([128, NT, ND, 128], BF16)  # LN2(x+h)^T spill

            # ============ Phase A: LN1, QKV, softmaxes, kv partials ==========
            with tc.tile_pool(name="wqkv", bufs=1) as wq, \
                 tc.tile_pool(name="pa", bufs=2) as pa, \
                 tc.tile_pool(name="pa_ps", bufs=3, space="PSUM") as pap, \
                 tc.tile_pool(name="kv_ps", bufs=1, space="PSUM") as kvp, \
                 tc.tile_pool(name="tp_ps", bufs=3, space="PSUM") as tpp:
                wqkv_sb = wq.tile([128, ND, F3], BF16)     # 6 MB
                for ds in range(ND):
                    nc.sync.dma_start(wqkv_sb[:, ds, :],
                                      wqkv_d[ds * 128:(ds + 1) * 128, :])

                kv_ps = kvp.tile([128, ND, DH + 1], F32)   # 16 heads as 8 pairs

                for t in range(NT):
                    x_t = pa.tile([128, D], F32, tag="xa")
                    nc.sync.dma_start(x_t[:], x_d[t * 128:(t + 1) * 128, :])
                    # LN1 statistics
                    st = pa.tile([128, 2, 6], F32, tag="st")
                    nc.vector.bn_stats(st[:, 0, :], x_t[:, 0:512])
                    nc.vector.bn_stats(st[:, 1, :], x_t[:, 512:1024])
                    mv = pa.tile([128, 2], F32, tag="mv")
                    nc.vector.bn_aggr(mv[:], st[:])
                    sd = pa.tile([128, 1], F32, tag="sd")
                    nc.scalar.activation(sd[:], mv[:, 1:2], ACTF.Sqrt, bias=eps_t[:])
                    rs = pa.tile([128, 1], F32, tag="rs")
                    nc.vector.reciprocal(rs[:], sd[:])
                    xn = pa.tile([128, D], F32R, tag="xn")
                    nc.vector.tensor_scalar_sub(xn[:], x_t[:], mv[:, 0:1])
                    nc.vector.tensor_scalar_mul(xn[:], xn[:], rs[:])
                    # transpose xn -> [d, t] slices (bf16 for the QKV matmul)
                    xnT = pa.tile([128, ND, 128], BF16, tag="xnT")
                    for ds in range(ND):
                        ps = tpp.tile([128, 128], F32R, tag="tp")
                        nc.tensor.transpose(ps[:], xn[:, ds * 128:(ds + 1) * 128],
                                            ident[:])
                        nc.vector.tensor_copy(xnT[:, ds, :], ps.bitcast(F32))

                    ek_all = pa.tile([128, H, DH], BF16, tag="ek")
                    # QKV projection, one 512-wide chunk at a time
                    for fc in range(F3 // 512):
                        ps = pap.tile([128, 512], F32, tag="mm")
                        for ds in range(ND):
                            nc.tensor.matmul(ps[:], xnT[:, ds, :],
                                             wqkv_sb[:, ds, fc * 512:(fc + 1) * 512],
                                             start=(ds == 0), stop=(ds == ND - 1))
                        if has_bqkv:
                            nc.vector.tensor_add(
                                ps[:], ps[:], bqkv_bc[:, fc * 512:(fc + 1) * 512])
                        h8 = fc % 2 * 8        # first head of this 512 chunk
                        if fc < 2:
                            # q chunk: feature softmax over 64-wide segments
                            eq = pa.tile([128, 8, DH], F32, tag="eq")
                            nc.scalar.activation(
                                eq.rearrange("p h d -> p (h d)"), ps[:], ACTF.Exp)
                            sq = pa.tile([128, 8], F32, tag="sq")
                            nc.vector.tensor_reduce(sq[:], eq[:], axis=AX.X,
                                                    op=ALU.add)
                            rq = pa.tile([128, 8], F32, tag="rq")
                            nc.vector.reciprocal(rq[:], sq[:])
                            qs = pa.tile([128, 8, DH], F32R, tag="qs")
                            nc.vector.tensor_mul(
                                qs[:], eq[:],
                                rq.unsqueeze(2).broadcast_to([128, 8, DH]))
                            qsf = qs.rearrange("p h d -> p (h d)")
                            for j in range(4):
                                ds = fc * 4 + j
                                ps2 = tpp.tile([128, 128], F32R, tag="tp")
                                nc.tensor.transpose(
                                    ps2[:], qsf[:, j * 128:(j + 1) * 128],
                                    ident[:])
                                nc.vector.tensor_copy(qT_all[:, t, ds, :],
                                                      ps2.bitcast(F32))
                        elif fc < 4:
                            # k chunk: exp
                            nc.scalar.activation(
                                ek_all[:, h8:h8 + 8, :].rearrange(
                                    "p h d -> p (h d)"), ps[:], ACTF.Exp)
                        else:
                            # v chunk: [v | 1], then kv partial matmuls
                            vp = pa.tile([128, 8, DH + 1], BF16, tag="vp")
                            nc.vector.tensor_copy(
                                vp[:, :, 0:DH],
                                ps.rearrange("p (h d) -> p h d", h=8))
                            nc.vector.tensor_copy(vp[:, :, DH:DH + 1],
                                                  ones8.unsqueeze(2))
                            ekf = ek_all.rearrange("p h d -> p (h d)")
                            for i in range(8):
                                h = h8 + i
                                hp, half = h // 2, h % 2
                                dst = kv_ps[half * 64:(half + 1) * 64, hp, :]
                                nc.tensor.matmul(
                                    dst, ekf[:, h * 64:(h + 1) * 64],
                                    vp[:, i, :],
                                    start=(t == 0), stop=(t == NT - 1),
                                    tile_position=(0, 64) if half else None)

                kv_sb = pa.tile([128, ND, DH + 1], F32, tag="kvsb")
                nc.vector.tensor_copy(kv_sb[:], kv_ps[:])
                nc.sync.dma_start(kv_in[:], kv_sb[:])
                nc.gpsimd.collective_compute(
                    "AllReduce", ALU.add,
                    ins=[kv_in.opt()], outs=[kv_out.opt()],
                    replica_groups=[[0, 1], [2, 3], [4, 5], [6, 7]],
                )
                nc.sync.dma_start(kv_red[:], kv_out[:])

            # ============ Phase B: attention out, out-proj, LN2 ==============
            with tc.tile_pool(name="wb", bufs=1) as wb, \
                 tc.tile_pool(name="pb", bufs=2) as pb, \
                 tc.tile_pool(name="attnT", bufs=1) as atp, \
                 tc.tile_pool(name="pb_ps", bufs=2, space="PSUM") as pbp, \
                 tc.tile_pool(name="tpb_ps", bufs=3, space="PSUM") as tpb:
                wout_sb = wb.tile([128, ND, D], F32R)      # 4 MB
                for ds in range(ND):
                    nc.sync.dma_start(wout_sb[:, ds, :],
                                      wout_d[ds * 128:(ds + 1) * 128, :])
                # normalize kv rows by sumexp
                rcp = pb.tile([128, ND], F32, tag="rcp")
                nc.vector.reciprocal(rcp[:], kv_red[:, :, DH])
                for hp in range(ND):
                    nc.vector.tensor_scalar_mul(kvn[:, hp, :], kv_red[:, hp, 0:DH],
                                                rcp[:, hp:hp + 1])
                # attention: attnT[e, t] per head (feature-major)
                attnT = atp.tile([128, ND, T], F32R)       # 8 MB
                for hp in range(ND):
                    for c4 in range(4):
                        ps = pbp.tile([128, 512], F32, tag="at")
                        for half in range(2):
                            sl = slice(half * 64, (half + 1) * 64)
                            nc.tensor.matmul(
                                ps[sl, :], kvn[sl, hp, :],
                                qT_all[sl, c4 * 4:(c4 + 1) * 4, hp, :],
                                start=True, stop=True,
                                tile_position=(64, 64) if half else None)
                        nc.vector.tensor_copy(attnT[:, hp, c4 * 512:(c4 + 1) * 512],
                                              ps[:])
                # out-projection + residual + LN2 + transpose z
                for t in range(NT):
                    x_t = pb.tile([128, D], F32, tag="xb")
                    nc.sync.dma_start(x_t[:], x_d[t * 128:(t + 1) * 128, :])
                    y = pb.tile([128, D], F32, tag="y")
                    for jc in range(2):
                        ps = pbp.tile([128, 512], F32, tag="op")
                        for es in range(ND):
                            nc.tensor.matmul(
                                ps[:], attnT[:, es, t * 128:(t + 1) * 128],
                                wout_sb[:, es, jc * 512:(jc + 1) * 512],
                                start=(es == 0), stop=(es == ND - 1))
                        if has_bout:
                            nc.vector.tensor_add(
                                y[:, jc * 512:(jc + 1) * 512], ps[:],
                                bout_bc[:, jc * 512:(jc + 1) * 512])
                        else:
                            nc.vector.tensor_copy(y[:, jc * 512:(jc + 1) * 512],
                                                  ps[:])
                    nc.vector.tensor_add(y[:], y[:], x_t[:])
                    st = pb.tile([128, 2, 6], F32, tag="st2")
                    nc.vector.bn_stats(st[:, 0, :], y[:, 0:512])
                    nc.vector.bn_stats(st[:, 1, :], y[:, 512:1024])
                    mv = pb.tile([128, 2], F32, tag="mv2")
                    nc.vector.bn_aggr(mv[:], st[:])
                    sd = pb.tile([128, 1], F32, tag="sd2")
                    nc.scalar.activation(sd[:], mv[:, 1:2], ACTF.Sqrt, bias=eps_t[:])
                    rs = pb.tile([128, 1], F32, tag="rs2")
                    nc.vector.reciprocal(rs[:], sd[:])
                    z = pb.tile([128, D], F32R, tag="z")
                    nc.vector.tensor_scalar_sub(z[:], y[:], mv[:, 0:1])
                    nc.vector.tensor_scalar_mul(z[:], z[:], rs[:])
                    zTs = pb.tile([128, ND, 128], BF16, tag="zTs")
                    for ds in range(ND):
                        ps = tpb.tile([128, 128], F32R, tag="tpb")
                        nc.tensor.transpose(ps[:], z[:, ds * 128:(ds + 1) * 128],
                                            ident[:])
                        nc.vector.tensor_copy(zTs[:, ds, :], ps.bitcast(F32))
                    nc.sync.dma_start(zT_dram[:, t, :, :], zTs[:])

            # ============ Phase C: FFN ======================================
            with tc.tile_pool(name="ws", bufs=2) as ws, \
                 tc.tile_pool(name="w2p", bufs=1) as w2p, \
                 tc.tile_pool(name="up", bufs=1) as up, \
                 tc.tile_pool(name="zc", bufs=2) as zc, \
                 tc.tile_pool(name="pc", bufs=2) as pc, \
                 tc.tile_pool(name="pc_ps", bufs=3, space="PSUM") as pcp:
                for c4 in range(4):
                    w2c = w2p.tile([128, NF, D], BF16, tag="w2")   # 8 MB
                    for fs in range(NF):
                        nc.sync.dma_start(w2c[:, fs, :],
                                          w2_d[fs * 128:(fs + 1) * 128, :])
                    zTc = zc.tile([128, 4, ND, 128], BF16, tag="zc")
                    nc.sync.dma_start(zTc[:], zT_dram[:, c4 * 4:(c4 + 1) * 4, :, :])
                    u_t = up.tile([128, NF, 512], BF16, tag="u")   # 4 MB
                    for ft in range(NF):
                        w1_t = ws.tile([128, ND, 128], BF16, tag="w1")
                        nc.sync.dma_start(
                            w1_t[:],
                            w1_d[:, ft * 128:(ft + 1) * 128].rearrange(
                                "(ds p) f -> p ds f", p=128))
                        ps = pcp.tile([128, 512], F32, tag="g")
                        for ds in range(ND):
                            nc.tensor.matmul(
                                ps[:], w1_t[:, ds, :], zTc[:, :, ds, :],
                                start=(ds == 0), stop=(ds == ND - 1))
                        nc.scalar.activation(u_t[:, ft, :], ps[:], ACTF.Gelu,
                                             bias=b1_cols[:, ft:ft + 1])
                    for tt in range(4):
                        t = c4 * 4 + tt
                        x_t = pc.tile([128, D], F32, tag="xc")
                        nc.sync.dma_start(x_t[:], x_d[t * 128:(t + 1) * 128, :])
                        o_t = pc.tile([128, D], F32, tag="o")
                        for jc in range(2):
                            ps2 = pcp.tile([128, 512], F32, tag="d")
                            for fs in range(NF):
                                nc.tensor.matmul(
                                    ps2[:], u_t[:, fs, tt * 128:(tt + 1) * 128],
                                    w2c[:, fs, jc * 512:(jc + 1) * 512],
                                    start=(fs == 0), stop=(fs == NF - 1))
                            if has_b2:
                                nc.vector.tensor_add(
                                    o_t[:, jc * 512:(jc + 1) * 512], ps2[:],
                                    b2_bc[:, jc * 512:(jc + 1) * 512])
                            else:
                                nc.vector.tensor_copy(
                                    o_t[:, jc * 512:(jc + 1) * 512], ps2[:])
                        nc.vector.tensor_add(o_t[:], o_t[:], x_t[:])
                        nc.sync.dma_start(out_d[t * 128:(t + 1) * 128, :], o_t[:])

    nc.compile()
    return nc


def kernel(x, w_qkv, b_qkv, w_attn_out, b_attn_out, w_ffn1, b_ffn1,
           w_ffn2, b_ffn2, g1, beta1, g2, beta2):
    x = np.asarray(x, dtype=np.float32)
    B, S, _ = x.shape
    xf = np.ascontiguousarray(x.reshape(-1, D))

    # fold LN gamma/beta into the following matmul weights
    wqkv_g = np.ascontiguousarray(
        (np.asarray(g1)[:, None] * np.asarray(w_qkv)).astype(ml_dtypes.bfloat16))
    bqkv_eff = (np.asarray(beta1) @ np.asarray(w_qkv)
                + np.asarray(b_qkv)).astype(np.float32).reshape(1, F3)
    w1g = np.ascontiguousarray(
        (np.asarray(g2)[:, None] * np.asarray(w_ffn1)).astype(ml_dtypes.bfloat16))
    b1_eff = (np.asarray(beta2) @ np.asarray(w_ffn1)
              + np.asarray(b_ffn1)).astype(np.float32).reshape(NF, 128)
    w2 = np.ascontiguousarray(np.asarray(w_ffn2).astype(ml_dtypes.bfloat16))
    wout = np.ascontiguousarray(np.asarray(w_attn_out).astype(np.float32))
    bout = np.asarray(b_attn_out).astype(np.float32).reshape(1, D)
    b2 = np.asarray(b_ffn2).astype(np.float32).reshape(1, D)

    flags = (bool(np.any(bqkv_eff)), bool(np.any(bout)), bool(np.any(b2)))
    if _CACHE.get("flags") != flags:
        _CACHE["nc"] = build_kernel(*flags)
        _CACHE["flags"] = flags
    nc = _CACHE["nc"]

    shared = {"wqkv": wqkv_g, "wout": wout, "w1": w1g, "w2": w2,
              "bqkv": bqkv_eff, "bout": bout, "b1": b1_eff, "b2": b2,
              "ident": np.eye(128, dtype=np.float32),
              "ones8": np.ones((1, 8), dtype=np.float32)}
    in_maps = [dict(x=np.ascontiguousarray(xf[c * T:(c + 1) * T]), **shared)
               for c in range(N_CORES)]

    res = bass_utils.run_bass_kernel_spmd(nc, in_maps,
                                          core_ids=list(range(N_CORES)))
    out = np.concatenate([res.results[c]["out"] for c in range(N_CORES)], axis=0)
    return out.reshape(B, S, D).astype(np.float32)


# revision 8
# speedup vs baseline: 1.0904x; 1.0904x over previous
"""Trainium2 Bass kernel for a dense transformer block with linear attention.

Reference computation (per batch b, sequence s):
    h   = linear_attention(LN1(x)); z = LN2(x + h)
    out = x + GeLU(z @ W1 + b1) @ W2 + b2

Sharding: tokens (B*S = 16384) split 8 ways -> 2048 tokens/core; each core
holds half the sequence of one batch element. The only cross-token coupling
is the sequence-dim softmax of k and the kv-einsum, handled by a pairwise
AllReduce of per-head [64, 65] (k^T v | sum exp k) statistics between the
two cores of each batch element.

LN gamma/beta are folded into the following matmul weights on the host.
Softmax max-subtraction is skipped (inputs are O(1); exp is safe in f32).
"""
import numpy as np
import ml_dtypes

import concourse.bass as bass
import concourse.tile as tile
from concourse import bacc, mybir
from concourse import bass_utils
from concourse.masks import make_identity

F32 = mybir.dt.float32
F32R = mybir.dt.float32r
BF16 = mybir.dt.bfloat16
AX = mybir.AxisListType
ALU = mybir.AluOpType
ACTF = mybir.ActivationFunctionType

N_CORES = 8
T = 2048          # tokens per core
D = 1024
H = 16            # heads
DH = 64
F3 = 3 * D        # qkv width
DFF = 4096
NT = T // 128     # 16 token tiles
ND = D // 128     # 8 d-slices
NF = DFF // 128   # 32 ff tiles
EPS = 1e-5

_CACHE = {}


def _r(ap):
    return ap.bitcast(F32R)


def build_kernel(has_bqkv, has_bout, has_b2):
    nc = bacc.Bacc("TRN2", target_bir_lowering=False, debug=False,
                   enable_asserts=True, num_devices=N_CORES)

    x_d = nc.dram_tensor("x", [T, D], F32, kind="ExternalInput").ap()
    wqkv_d = nc.dram_tensor("wqkv", [D, F3], BF16, kind="ExternalInput").ap()
    wout_d = nc.dram_tensor("wout", [D, D], F32R, kind="ExternalInput").ap()
    w1_d = nc.dram_tensor("w1", [D, DFF], BF16, kind="ExternalInput").ap()
    w2_d = nc.dram_tensor("w2", [DFF, D], BF16, kind="ExternalInput").ap()
    bqkv_d = nc.dram_tensor("bqkv", [1, F3], F32, kind="ExternalInput").ap()
    ident_d = nc.dram_tensor("ident", [128, 128], F32R, kind="ExternalInput").ap()
    ones_d = nc.dram_tensor("ones8", [1, 8], F32, kind="ExternalInput").ap()
    bout_d = nc.dram_tensor("bout", [1, D], F32, kind="ExternalInput").ap()
    b1_d = nc.dram_tensor("b1", [NF, 128], F32, kind="ExternalInput").ap()
    b2_d = nc.dram_tensor("b2", [1, D], F32, kind="ExternalInput").ap()
    out_d = nc.dram_tensor("out", [T, D], F32, kind="ExternalOutput").ap()

    with tile.TileContext(nc) as tc:
        with tc.tile_pool(name="const", bufs=1) as cst, \
             tc.tile_pool(name="persist", bufs=1) as per, \
             tc.tile_pool(name="dram", bufs=1, space="DRAM") as dramp:
            ident = cst.tile([128, 128], F32R)
            nc.sync.dma_start(ident[:], ident_d[:])
            ones8 = cst.tile([128, 8], F32)
            nc.sync.dma_start(ones8[:], ones_d.partition_broadcast(128).squeeze(1))
            eps_t = cst.tile([128, 1], F32)
            nc.vector.memset(eps_t[:], EPS)
            b1_cols = cst.tile([128, NF], F32)
            nc.sync.dma_start(b1_cols[:], b1_d.transpose([1, 0]))
            bqkv_bc = bout_bc = b2_bc = None
            if has_bqkv:
                bqkv_bc = cst.tile([128, F3], F32)
                nc.sync.dma_start(bqkv_bc[:],
                                  bqkv_d.partition_broadcast(128).squeeze(1))
            if has_bout:
                bout_bc = cst.tile([128, D], F32)
                nc.sync.dma_start(bout_bc[:],
                                  bout_d.partition_broadcast(128).squeeze(1))
            if has_b2:
                b2_bc = cst.tile([128, D], F32)
                nc.sync.dma_start(b2_bc[:],
                                  b2_d.partition_broadcast(128).squeeze(1))

            qT_all = per.tile([128, NT, ND, 128], BF16)    # 4 MB, softmaxed q^T
            kv_red = per.tile([128, ND, DH + 1], F32)      # allreduced kv | sumexp
            kvn = per.tile([128, ND, DH], BF16)            # normalized kv

            kv_in = dramp.tile([128, ND, DH + 1], F32)
            kv_out = dramp.tile([128, ND, DH + 1], F32)
            zT_dram = dramp.tile([128, NT, ND, 128], BF16)  # LN2(x+h)^T spill

            # ============ Phase A: LN1, QKV, softmaxes, kv partials ==========
            with tc.tile_pool(name="wqkv", bufs=1) as wq, \
                 tc.tile_pool(name="pa", bufs=3) as pa, \
                 tc.tile_pool(name="pa_ps", bufs=3, space="PSUM") as pap, \
                 tc.tile_pool(name="kv_ps", bufs=1, space="PSUM") as kvp, \
                 tc.tile_pool(name="tp_ps", bufs=3, space="PSUM") as tpp:
                wqkv_sb = wq.tile([128, ND, F3], BF16)     # 6 MB
                for ds in range(ND):
                    nc.sync.dma_start(wqkv_sb[:, ds, :],
                                      wqkv_d[ds * 128:(ds + 1) * 128, :])

                kv_ps = kvp.tile([128, ND, DH + 1], F32)   # 16 heads as 8 pairs

                for t in range(NT):
                    x_t = pa.tile([128, D], F32, tag="xa")
                    nc.sync.dma_start(x_t[:], x_d[t * 128:(t + 1) * 128, :])
                    # LN1 statistics
                    st = pa.tile([128, 2, 6], F32, tag="st")
                    nc.vector.bn_stats(st[:, 0, :], x_t[:, 0:512])
                    nc.vector.bn_stats(st[:, 1, :], x_t[:, 512:1024])
                    mv = pa.tile([128, 2], F32, tag="mv")
                    nc.vector.bn_aggr(mv[:], st[:])
                    sd = pa.tile([128, 1], F32, tag="sd")
                    nc.scalar.activation(sd[:], mv[:, 1:2], ACTF.Sqrt, bias=eps_t[:])
                    rs = pa.tile([128, 1], F32, tag="rs")
                    nc.vector.reciprocal(rs[:], sd[:])
                    xn = pa.tile([128, D], F32R, tag="xn")
                    nc.vector.tensor_scalar_sub(xn[:], x_t[:], mv[:, 0:1])
                    nc.vector.tensor_scalar_mul(xn[:], xn[:], rs[:])
                    # transpose xn -> [d, t] slices (bf16 for the QKV matmul)
                    xnT = pa.tile([128, ND, 128], BF16, tag="xnT")
                    for ds in range(ND):
                        ps = tpp.tile([128, 128], F32R, tag="tp")
                        nc.tensor.transpose(ps[:], xn[:, ds * 128:(ds + 1) * 128],
                                            ident[:])
                        nc.vector.tensor_copy(xnT[:, ds, :], ps.bitcast(F32))

                    ek_all = pa.tile([128, H, DH], BF16, tag="ek")
                    # QKV projection, one 512-wide chunk at a time
                    for fc in range(F3 // 512):
                        ps = pap.tile([128, 512], F32, tag="mm")
                        for ds in range(ND):
                            nc.tensor.matmul(ps[:], xnT[:, ds, :],
                                             wqkv_sb[:, ds, fc * 512:(fc + 1) * 512],
                                             start=(ds == 0), stop=(ds == ND - 1))
                        if has_bqkv:
                            nc.vector.tensor_add(
                                ps[:], ps[:], bqkv_bc[:, fc * 512:(fc + 1) * 512])
                        h8 = fc % 2 * 8        # first head of this 512 chunk
                        if fc < 2:
                            # q chunk: feature softmax over 64-wide segments
                            eq = pa.tile([128, 8, DH], F32, tag="eq")
                            nc.scalar.activation(
                                eq.rearrange("p h d -> p (h d)"), ps[:], ACTF.Exp)
                            sq = pa.tile([128, 8], F32, tag="sq")
                            nc.vector.tensor_reduce(sq[:], eq[:], axis=AX.X,
                                                    op=ALU.add)
                            rq = pa.tile([128, 8], F32, tag="rq")
                            nc.vector.reciprocal(rq[:], sq[:])
                            qs = pa.tile([128, 8, DH], F32R, tag="qs")
                            nc.vector.tensor_mul(
                                qs[:], eq[:],
                                rq.unsqueeze(2).broadcast_to([128, 8, DH]))
                            qsf = qs.rearrange("p h d -> p (h d)")
                            for j in range(4):
                                ds = fc * 4 + j
                                ps2 = tpp.tile([128, 128], F32R, tag="tp")
                                nc.tensor.transpose(
                                    ps2[:], qsf[:, j * 128:(j + 1) * 128],
                                    ident[:])
                                nc.vector.tensor_copy(qT_all[:, t, ds, :],
                                                      ps2.bitcast(F32))
                        elif fc < 4:
                            # k chunk: exp
                            nc.scalar.activation(
                                ek_all[:, h8:h8 + 8, :].rearrange(
                                    "p h d -> p (h d)"), ps[:], ACTF.Exp)
                        else:
                            # v chunk: [v | 1], then kv partial matmuls
                            vp = pa.tile([128, 8, DH + 1], BF16, tag="vp")
                            nc.vector.tensor_copy(
                                vp[:, :, 0:DH],
                                ps.rearrange("p (h d) -> p h d", h=8))
                            nc.vector.tensor_copy(vp[:, :, DH:DH + 1],
                                                  ones8.unsqueeze(2))
                            ekf = ek_all.rearrange("p h d -> p (h d)")
                            for i in range(8):
                                h = h8 + i
                                hp, half = h // 2, h % 2
                                dst = kv_ps[half * 64:(half + 1) * 64, hp, :]
                                nc.tensor.matmul(
                                    dst, ekf[:, h * 64:(h + 1) * 64],
                                    vp[:, i, :],
                                    start=(t == 0), stop=(t == NT - 1),
                                    tile_position=(0, 64) if half else None)

                kv_sb = pa.tile([128, ND, DH + 1], F32, tag="kvsb")
                nc.vector.tensor_copy(kv_sb[:], kv_ps[:])
                nc.sync.dma_start(kv_in[:], kv_sb[:])
                nc.gpsimd.collective_compute(
                    "AllReduce", ALU.add,
                    ins=[kv_in.opt()], outs=[kv_out.opt()],
                    replica_groups=[[0, 1], [2, 3], [4, 5], [6, 7]],
                )
                nc.sync.dma_start(kv_red[:], kv_out[:])

            # ============ Phase B: attention out, out-proj, LN2 ==============
            with tc.tile_pool(name="wb", bufs=1) as wb, \
                 tc.tile_pool(name="pb", bufs=3) as pb, \
                 tc.tile_pool(name="attnT", bufs=1) as atp, \
                 tc.tile_pool(name="pb_ps", bufs=2, space="PSUM") as pbp, \
                 tc.tile_pool(name="tpb_ps", bufs=3, space="PSUM") as tpb:
                wout_sb = wb.tile([128, ND, D], F32R)      # 4 MB
                for ds in range(ND):
                    nc.sync.dma_start(wout_sb[:, ds, :],
                                      wout_d[ds * 128:(ds + 1) * 128, :])
                # normalize kv rows by sumexp
                rcp = pb.tile([128, ND], F32, tag="rcp")
                nc.vector.reciprocal(rcp[:], kv_red[:, :, DH])
                for hp in range(ND):
                    nc.vector.tensor_scalar_mul(kvn[:, hp, :], kv_red[:, hp, 0:DH],
                                                rcp[:, hp:hp + 1])
                # attention: attnT[e, t] per head (feature-major)
                attnT = atp.tile([128, ND, T], F32R)       # 8 MB
                for hp in range(ND):
                    for c4 in range(4):
                        ps = pbp.tile([128, 512], F32, tag="at")
                        for half in range(2):
                            sl = slice(half * 64, (half + 1) * 64)
                            nc.tensor.matmul(
                                ps[sl, :], kvn[sl, hp, :],
                                qT_all[sl, c4 * 4:(c4 + 1) * 4, hp, :],
                                start=True, stop=True,
                                tile_position=(64, 64) if half else None)
                        nc.vector.tensor_copy(attnT[:, hp, c4 * 512:(c4 + 1) * 512],
                                              ps[:])
                # out-projection + residual + LN2 + transpose z
                for t in range(NT):
                    x_t = pb.tile([128, D], F32, tag="xb")
                    nc.sync.dma_start(x_t[:], x_d[t * 128:(t + 1) * 128, :])
                    y = pb.tile([128, D], F32, tag="y")
                    for jc in range(2):
                        ps = pbp.tile([128, 512], F32, tag="op")
                        for es in range(ND):
                            nc.tensor.matmul(
                                ps[:], attnT[:, es, t * 128:(t + 1) * 128],
                                wout_sb[:, es, jc * 512:(jc + 1) * 512],
                                start=(es == 0), stop=(es == ND - 1))
                        if has_bout:
                            nc.vector.tensor_add(
                                y[:, jc * 512:(jc + 1) * 512], ps[:],
                                bout_bc[:, jc * 512:(jc + 1) * 512])
                        else:
                            nc.vector.tensor_copy(y[:, jc * 512:(jc + 1) * 512],
                                                  ps[:])
                    nc.vector.tensor_add(y[:], y[:], x_t[:])
                    st = pb.tile([128, 2, 6], F32, tag="st2")
                    nc.vector.bn_stats(st[:, 0, :], y[:, 0:512])
                    nc.vector.bn_stats(st[:, 1, :], y[:, 512:1024])
                    mv = pb.tile([128, 2], F32, tag="mv2")
                    nc.vector.bn_aggr(mv[:], st[:])
                    sd = pb.tile([128, 1], F32, tag="sd2")
                    nc.scalar.activation(sd[:], mv[:, 1:2], ACTF.Sqrt, bias=eps_t[:])
                    rs = pb.tile([128, 1], F32, tag="rs2")
                    nc.vector.reciprocal(rs[:], sd[:])
                    z = pb.tile([128, D], F32R, tag="z")
                    nc.vector.tensor_scalar_sub(z[:], y[:], mv[:, 0:1])
                    nc.vector.tensor_scalar_mul(z[:], z[:], rs[:])
                    zTs = pb.tile([128, ND, 128], BF16, tag="zTs")
                    for ds in range(ND):
                        ps = tpb.tile([128, 128], F32R, tag="tpb")
                        nc.tensor.transpose(ps[:], z[:, ds * 128:(ds + 1) * 128],
                                            ident[:])
                        nc.vector.tensor_copy(zTs[:, ds, :], ps.bitcast(F32))
                    nc.sync.dma_start(zT_dram[:, t, :, :], zTs[:])

            # ============ Phase C: FFN ======================================
            with tc.tile_pool(name="ws", bufs=2) as ws, \
                 tc.tile_pool(name="w2p", bufs=1) as w2p, \
                 tc.tile_pool(name="up", bufs=1) as up, \
                 tc.tile_pool(name="zc", bufs=2) as zc, \
                 tc.tile_pool(name="pc", bufs=3) as pc, \
                 tc.tile_pool(name="pc_ps", bufs=3, space="PSUM") as pcp:
                for c4 in range(4):
                    w2c = w2p.tile([128, NF, D], BF16, tag="w2")   # 8 MB
                    for fs in range(NF):
                        nc.sync.dma_start(w2c[:, fs, :],
                                          w2_d[fs * 128:(fs + 1) * 128, :])
                    zTc = zc.tile([128, 4, ND, 128], BF16, tag="zc")
                    nc.sync.dma_start(zTc[:], zT_dram[:, c4 * 4:(c4 + 1) * 4, :, :])
                    u_t = up.tile([128, NF, 512], BF16, tag="u")   # 4 MB
                    for ft in range(NF):
                        w1_t = ws.tile([128, ND, 128], BF16, tag="w1")
                        nc.sync.dma_start(
                            w1_t[:],
                            w1_d[:, ft * 128:(ft + 1) * 128].rearrange(
                                "(ds p) f -> p ds f", p=128))
                        ps = pcp.tile([128, 512], F32, tag="g")
                        for ds in range(ND):
                            nc.tensor.matmul(
                                ps[:], w1_t[:, ds, :], zTc[:, :, ds, :],
                                start=(ds == 0), stop=(ds == ND - 1))
                        nc.scalar.activation(u_t[:, ft, :], ps[:], ACTF.Gelu,
                                             bias=b1_cols[:, ft:ft + 1])
                    for tt in range(4):
                        t = c4 * 4 + tt
                        x_t = pc.tile([128, D], F32, tag="xc")
                        nc.sync.dma_start(x_t[:], x_d[t * 128:(t + 1) * 128, :])
                        o_t = pc.tile([128, D], F32, tag="o")
                        for jc in range(2):
                            ps2 = pcp.tile([128, 512], F32, tag="d")
                            for fs in range(NF):
                                nc.tensor.matmul(
                                    ps2[:], u_t[:, fs, tt * 128:(tt + 1) * 128],
                                    w2c[:, fs, jc * 512:(jc + 1) * 512],
                                    start=(fs == 0), stop=(fs == NF - 1))
                            if has_b2:
                                nc.vector.tensor_add(
                                    o_t[:, jc * 512:(jc + 1) * 512], ps2[:],
                                    b2_bc[:, jc * 512:(jc + 1) * 512])
                            else:
                                nc.vector.tensor_copy(
                                    o_t[:, jc * 512:(jc + 1) * 512], ps2[:])
                        nc.vector.tensor_add(o_t[:], o_t[:], x_t[:])
                        nc.sync.dma_start(out_d[t * 128:(t + 1) * 128, :], o_t[:])

    nc.compile()
    return nc


def kernel(x, w_qkv, b_qkv, w_attn_out, b_attn_out, w_ffn1, b_ffn1,
           w_ffn2, b_ffn2, g1, beta1, g2, beta2):
    x = np.asarray(x, dtype=np.float32)
    B, S, _ = x.shape
    xf = np.ascontiguousarray(x.reshape(-1, D))

    # fold LN gamma/beta into the following matmul weights
    wqkv_g = np.ascontiguousarray(
        (np.asarray(g1)[:, None] * np.asarray(w_qkv)).astype(ml_dtypes.bfloat16))
    bqkv_eff = (np.asarray(beta1) @ np.asarray(w_qkv)
                + np.asarray(b_qkv)).astype(np.float32).reshape(1, F3)
    w1g = np.ascontiguousarray(
        (np.asarray(g2)[:, None] * np.asarray(w_ffn1)).astype(ml_dtypes.bfloat16))
    b1_eff = (np.asarray(beta2) @ np.asarray(w_ffn1)
              + np.asarray(b_ffn1)).astype(np.float32).reshape(NF, 128)
    w2 = np.ascontiguousarray(np.asarray(w_ffn2).astype(ml_dtypes.bfloat16))
    wout = np.ascontiguousarray(np.asarray(w_attn_out).astype(np.float32))
    bout = np.asarray(b_attn_out).astype(np.float32).reshape(1, D)
    b2 = np.asarray(b_ffn2).astype(np.float32).reshape(1, D)

    flags = (bool(np.any(bqkv_eff)), bool(np.any(bout)), bool(np.any(b2)))
    if _CACHE.get("flags") != flags:
        _CACHE["nc"] = build_kernel(*flags)
        _CACHE["flags"] = flags
    nc = _CACHE["nc"]

    shared = {"wqkv": wqkv_g, "wout": wout, "w1": w1g, "w2": w2,
              "bqkv": bqkv_eff, "bout": bout, "b1": b1_eff, "b2": b2,
              "ident": np.eye(128, dtype=np.float32),
              "ones8": np.ones((1, 8), dtype=np.float32)}
    in_maps = [dict(x=np.ascontiguousarray(xf[c * T:(c + 1) * T]), **shared)
               for c in range(N_CORES)]

    res = bass_utils.run_bass_kernel_spmd(nc, in_maps,
                                          core_ids=list(range(N_CORES)))
    out = np.concatenate([res.results[c]["out"] for c in range(N_CORES)], axis=0)
    return out.reshape(B, S, D).astype(np.float32)
